# revision 17
# baseline (speedup 1.0000x reference)
"""Trainium2 Bass kernel for nn_Block_14499809591408 (sparse_attention).

Block: LN -> time_pool(ident/SoftPrefixMax/Shift1/Shift2) -> merged in_proj
-> causal+ALiBi attention (8 heads, d=64) + gelu FF (exp 4) -> merged out_proj.

Sharding (8 cores): data-parallel over batch (2) x tensor-parallel 4-way.
Core c: batch c//4, r = c%4, heads (r, r+4), FF p-slice r (512 of 2048 cols).
Each core computes a rank-640 partial of the output; host sums 4 partials
per batch and adds b_out.

Key kernel tricks:
- time_pool shifts are NOT materialized: in_proj reads shifted APs of the
  transposed activations (k-block 3 split into two K=64 matmuls).
- SoftPrefixMax: exp(5*xn-15) of the 64 spm channels transposed via PE,
  cumsum along free dim via tensor_tensor_scan, Ln; the /5 and +3 are
  folded into pre-scaled w_in rows + a per-out-column bias applied in the
  PSUM->SBUF epilogues.
- Attention in S.T layout [keys, queries]: K=65 augmented matmul adds the
  -8*slope*i per-query alibi shift (softmax shift-invariance makes its
  exact value uncritical); +slope*j rides the exp bias per partition.
  Causal mask inside diagonal 128x128 blocks via a 0/1 multiply after exp.
- ALiBi slopes decay fast: heads 0-3 only attend ~688 back, so their
  far j-blocks are skipped entirely (contribution < e^-25).
- AV matmul packs the softmax denominator as a 65th output row
  (ones-vector matmul in PSUM column strip 64 via tile_position).
"""

import math

import numpy as np
import ml_dtypes

import concourse.bass as bass
import concourse.bacc as bacc
import concourse.tile as tile
import concourse.mybir as mybir
from concourse.bass_utils import run_bass_kernel_spmd

BF16 = mybir.dt.bfloat16
F32 = mybir.dt.float32
AF = mybir.ActivationFunctionType
ALU = mybir.AluOpType

B, L, D = 2, 2048, 512
HEADS, DH, EXPF = 8, 64, 4
PSLICE = D * EXPF // 4          # 512 p-cols per core
NT = L // 128                   # 16 row/col blocks
NIC = L // 512                  # 4 query chunks
NKB = D // 128                  # 4 hidden k-blocks
SPM_LO, SPM_HI = 5 * (D // 8), 6 * (D // 8)   # 320, 384

# per-slot band threshold in key-positions; slot0 holds heads 0-3
# (slope >= 1/16 -> exp(-688/16) ~ e^-43), slot1 heads 4-7 needs full causal.
BAND = [688, 1 << 30]

nbf = ml_dtypes.bfloat16

# CoreSim doesn't implement Gelu; tests set this to validate everything else.
SIM_COMPAT = False


def _get_slopes(heads):
    def pow2(n):
        start = 2 ** (-(2 ** (-(math.log2(n) - 3))))
        return [start * start**i for i in range(n)]
    if math.log2(heads).is_integer():
        return pow2(heads)
    c = 2 ** math.floor(math.log2(heads))
    return pow2(c) + pow2(2 * c)[0::2][: heads - c]

SLOPES = _get_slopes(HEADS)


def _jb_range(ic, band):
    """Key-blocks [jb0, jb1) contributing to query chunk ic under banding."""
    jb0 = max(0, -(-(512 * ic - 127 - band) // 128))
    return jb0, 4 * ic + 4


def build_program(repeat=1):
    nc = bacc.Bacc("TRN2", debug=False)

    x_in = nc.dram_tensor("x", [L, D], F32, kind="ExternalInput").ap()
    w_in = nc.dram_tensor("w_in", [D, 896], BF16, kind="ExternalInput").ap()
    w_out = nc.dram_tensor("w_out", [640, D], BF16, kind="ExternalInput").ap()
    qaug_in = nc.dram_tensor("qaug", [3, L], BF16, kind="ExternalInput").ap()
    expb_in = nc.dram_tensor("expb", [128, 32], F32, kind="ExternalInput").ap()
    biasin_in = nc.dram_tensor("biasin", [128, 7], F32, kind="ExternalInput").ap()
    edgec_in = nc.dram_tensor("edgec", [128, 14], F32, kind="ExternalInput").ap()
    g5b5_in = nc.dram_tensor("g5b5", [64, 2], F32, kind="ExternalInput").ap()
    tri_in = nc.dram_tensor("tri01", [128, 128], BF16, kind="ExternalInput").ap()
    oscr = nc.dram_tensor("oscr", [8, 512], F32).ap()
    y_out = nc.dram_tensor("y", [L, D], F32, kind="ExternalOutput").ap()

    with tile.TileContext(nc) as tc:
        for _ in range(repeat):
            _body(tc, nc, x_in, w_in, w_out, qaug_in, expb_in, biasin_in,
                  edgec_in, g5b5_in, tri_in, oscr, y_out)
    nc.compile()
    return nc


def _body(tc, nc, x_in, w_in, w_out, qaug_in, expb_in, biasin_in,
          edgec_in, g5b5_in, tri_in, oscr, y_out):
    from contextlib import ExitStack
    ctx = ExitStack()
    consts = ctx.enter_context(tc.tile_pool(name="consts", bufs=1))
    persist = ctx.enter_context(tc.tile_pool(name="persist", bufs=1))
    xpool = ctx.enter_context(tc.tile_pool(name="xpool", bufs=16))
    xnpool = ctx.enter_context(tc.tile_pool(name="xnpool", bufs=6))
    spool = ctx.enter_context(tc.tile_pool(name="spool", bufs=3))
    ppool = ctx.enter_context(tc.tile_pool(name="ppool", bufs=6))
    opool = ctx.enter_context(tc.tile_pool(name="opool", bufs=3))
    ypool = ctx.enter_context(tc.tile_pool(name="ypool", bufs=3))
    pp_in = ctx.enter_context(tc.tile_pool(name="pp_in", bufs=2, space="PSUM"))
    pp_st = ctx.enter_context(tc.tile_pool(name="pp_st", bufs=2, space="PSUM"))
    pp_o = ctx.enter_context(tc.tile_pool(name="pp_o", bufs=2, space="PSUM"))

    # ---- constants ----
    expb = consts.tile([128, 32], F32)
    nc.sync.dma_start(out=expb, in_=expb_in)
    biasin = consts.tile([128, 7], F32)
    nc.sync.dma_start(out=biasin, in_=biasin_in)
    edgec = consts.tile([128, 14], F32)
    nc.sync.dma_start(out=edgec, in_=edgec_in)
    g5b5 = consts.tile([64, 2], F32)
    nc.sync.dma_start(out=g5b5, in_=g5b5_in)
    tri01 = consts.tile([128, 128], BF16)
    nc.sync.dma_start(out=tri01, in_=tri_in)
    ones1 = consts.tile([128, 1], BF16)
    nc.vector.memset(ones1, 1.0)
    eps128 = consts.tile([128, 1], F32)
    nc.vector.memset(eps128, 1e-5)
    zero128 = consts.tile([128, 1], F32)
    nc.vector.memset(zero128, 0.0)

    w_in_t = [persist.tile([128, 896], BF16, tag=f"w_in{k}", name=f"w_in{k}") for k in range(NKB)]
    for k in range(NKB):
        nc.sync.dma_start(out=w_in_t[k], in_=w_in[128 * k:128 * (k + 1), :])
    w_out_t = [persist.tile([128, 512], BF16, tag=f"w_out{k}", name=f"w_out{k}") for k in range(5)]
    for k in range(5):
        nc.sync.dma_start(out=w_out_t[k], in_=w_out[128 * k:128 * (k + 1), :])

    # ---- persistent activations ----
    xtT = [persist.tile([128, L], BF16, tag=f"xtT{k}", name=f"xtT{k}") for k in range(NKB)]
    xnT3 = persist.tile([128, L], BF16, tag="xnT3")
    expT = persist.tile([64, L], F32, tag="expT")
    cumT = persist.tile([64, L], F32, tag="cumT")
    qaugT = [persist.tile([65, L], BF16, tag=f"qaugT{s}", name=f"qaugT{s}") for s in range(2)]
    kaugT = [persist.tile([65, L], BF16, tag=f"kaugT{s}", name=f"kaugT{s}") for s in range(2)]
    vT = persist.tile([128, L], BF16, tag="vT")
    vrow = [persist.tile([128, NT * 64], BF16, tag=f"vrow{s}", name=f"vrow{s}") for s in range(2)]
    pT = [persist.tile([128, L], BF16, tag=f"pT{k}", name=f"pT{k}") for k in range(4)]
    oT = persist.tile([128, L], BF16, tag="oT")
    mv_all = persist.tile([128, 2 * NT], F32, tag="mv")
    rstd = persist.tile([128, NT], F32, tag="rstd")

    # ---- phase A: load x, LN stats ----
    x_t, xn_t = [], []
    for t in range(NT):
        xt = xpool.tile([128, D], F32, tag="x")
        nc.sync.dma_start(out=xt, in_=x_in[128 * t:128 * (t + 1), :])
        x_t.append(xt)
        stats = spool.tile([128, 6], F32, tag="stats", bufs=16)
        nc.vector.bn_stats(out=stats, in_=xt)
        nc.vector.bn_aggr(out=mv_all[:, 2 * t:2 * t + 2], in_=stats)
        if t % 4 == 3:
            g = t // 4
            lnv = spool.tile([128, 4], F32, tag="lnv")
            nc.scalar.activation(out=lnv, in_=mv_all[:, 8 * g + 1:8 * (g + 1):2],
                                 func=AF.Ln, bias=eps128, scale=1.0)
            nc.scalar.activation(out=rstd[:, 4 * g:4 * (g + 1)], in_=lnv,
                                 func=AF.Exp, bias=zero128, scale=-0.5)
    # normalize (raw: gamma folded into w_in host-side)
    for t in range(NT):
        xn = xnpool.tile([128, D], BF16, tag="xn")
        nc.vector.tensor_scalar(out=xn, in0=x_t[t],
                                scalar1=mv_all[:, 2 * t:2 * t + 1],
                                scalar2=rstd[:, t:t + 1],
                                op0=ALU.subtract, op1=ALU.mult)
        xn_t.append(xn)
    # transpose xn -> xtT (dma xbar); k-block 3 goes to scratch, then
    # shift-by-1 (rows 0:64) / shift-by-2 (rows 64:128) into xtT[3]
    for t in range(NT):
        for k in range(NKB):
            dst = xnT3 if k == 3 else xtT[k]
            nc.sync.dma_start(out=dst[:, 128 * t:128 * (t + 1)],
                              in_=xn_t[t][:, 128 * k:128 * (k + 1)],
                              transpose=True)
    nc.vector.memset(xtT[3][0:64, 0:1], 0.0)
    nc.vector.memset(xtT[3][64:128, 0:2], 0.0)
    nc.vector.tensor_copy(out=xtT[3][0:64, 1:L], in_=xnT3[0:64, 0:L - 1])
    nc.vector.tensor_copy(out=xtT[3][64:128, 2:L], in_=xnT3[64:128, 0:L - 2])
    # spm: exp(g5*xn+b5) of transposed spm channels, cumsum, Ln
    nc.scalar.activation(out=expT, in_=xtT[2][64:128, :], func=AF.Exp,
                         scale=g5b5[:, 0:1], bias=g5b5[:, 1:2])
    nc.vector.tensor_tensor_scan(out=cumT, data0=expT, data1=expT,
                                 initial=0.0, op0=ALU.add, op1=ALU.bypass)
    nc.scalar.activation(out=xtT[2][64:128, :], in_=cumT, func=AF.Ln,
                         bias=zero128[0:64, :], scale=1.0)

    # ---- phase B: in_proj ----
    # out-col blocks: 0=q(2 heads), 1=k, 2=v, 3..6=p
    def in_proj_mm(psum, oblk, ic):
        c0, c1 = 128 * oblk, 128 * (oblk + 1)
        i0 = 512 * ic
        for k in range(NKB):
            nc.tensor.matmul(psum, w_in_t[k][:, c0:c1],
                             xtT[k][:, i0:i0 + 512],
                             start=(k == 0), stop=(k == NKB - 1))

    for s in range(2):
        nc.sync.dma_start(out=qaugT[s][64:65, :], in_=qaug_in[s:s + 1, :])
        nc.sync.dma_start(out=kaugT[s][64:65, :], in_=qaug_in[2:3, :])

    # p-blocks (gelu) first so the ACT stream groups by table set:
    # [rstd Ln/Exp][spm Exp/Ln][gelu x16][attention exp ...] = 3 table loads
    for oblk in (3, 4, 5, 6, 0, 1, 2):
        for ic in range(NIC):
            psum = pp_in.tile([128, 512], F32, tag="inp")
            in_proj_mm(psum, oblk, ic)
            i0 = 512 * ic
            bia = biasin[:, oblk:oblk + 1]
            if ic == 0:
                nc.vector.tensor_tensor(
                    out=psum[:, 0:2], in0=psum[:, 0:2],
                    in1=edgec[:, 2 * oblk:2 * oblk + 2], op=ALU.subtract)
            if oblk == 0 or oblk == 1:
                dst = qaugT if oblk == 0 else kaugT
                for s in range(2):
                    nc.vector.tensor_scalar(
                        out=dst[s][0:64, i0:i0 + 512], in0=psum[64 * s:64 * (s + 1), :],
                        scalar1=bia[64 * s:64 * (s + 1), :], scalar2=None,
                        op0=ALU.add)
            elif oblk == 2:
                nc.vector.tensor_scalar(out=vT[:, i0:i0 + 512], in0=psum,
                                        scalar1=bia, scalar2=None, op0=ALU.add)
            else:
                gfun = AF.Identity if SIM_COMPAT else AF.Gelu
                nc.scalar.activation(out=pT[oblk - 3][:, i0:i0 + 512], in_=psum,
                                     func=gfun, bias=bia, scale=1.0)

    # v into row layout, per head: vrow[s][:, 64*jb:64*jb+64] = v[jb-block].T
    for jb in range(NT):
        for s in range(2):
            nc.sync.dma_start(out=vrow[s][:, 64 * jb:64 * (jb + 1)],
                              in_=vT[64 * s:64 * (s + 1), 128 * jb:128 * (jb + 1)],
                              transpose=True)

    # ---- phase C: attention per slot ----
    for s in range(2):
        for ic in range(NIC):
            opsum = pp_o.tile([65, 512], F32, tag="ops")
            jb0, jb1 = _jb_range(ic, BAND[s])
            for jb in range(jb0, jb1):
                mp = jb - 4 * ic
                col0 = 128 * mp if mp > 0 else 0
                wd = 512 - col0
                i0 = 512 * ic + col0
                st = pp_st.tile([128, 512], F32, tag="st")
                nc.tensor.matmul(st[:, 0:wd], kaugT[s][:, 128 * jb:128 * (jb + 1)],
                                 qaugT[s][:, i0:i0 + wd], start=True, stop=True)
                pt = ppool.tile([128, 512], BF16, tag="pt")
                nc.scalar.activation(out=pt[:, 0:wd], in_=st[:, 0:wd], func=AF.Exp,
                                     scale=0.125,
                                     bias=expb[:, 16 * s + jb:16 * s + jb + 1])
                if mp >= 0:
                    nc.vector.tensor_mul(pt[:, 0:128], pt[:, 0:128], tri01)
                first, last = jb == jb0, jb == jb1 - 1
                nc.tensor.matmul(opsum[0:64, col0:512],
                                 vrow[s][:, 64 * jb:64 * (jb + 1)],
                                 pt[:, 0:wd], start=first, stop=last)
                nc.tensor.matmul(opsum[64:65, col0:512], ones1, pt[:, 0:wd],
                                 start=first, stop=last,
                                 tile_position=(0, 64), skip_group_check=True)
            rl = spool.tile([1, 512], F32, tag="rl")
            nc.vector.reciprocal(out=rl, in_=opsum[64:65, :])
            slot = 4 * s + ic
            nc.sync.dma_start(out=oscr[slot:slot + 1, :], in_=rl)
            rb = opool.tile([64, 512], F32, tag="rb")
            nc.sync.dma_start(out=rb,
                              in_=oscr[slot:slot + 1, :].to_broadcast((64, 512)))
            nc.vector.tensor_mul(oT[64 * s:64 * (s + 1), 512 * ic:512 * (ic + 1)],
                                 opsum[0:64, :], rb)

    # ---- phase D: out_proj ----
    cT = [oT] + pT
    for t in range(NT):
        ypsum = pp_in.tile([128, 512], F32, tag="inp")
        for k in range(5):
            nc.tensor.matmul(ypsum, cT[k][:, 128 * t:128 * (t + 1)], w_out_t[k],
                             start=(k == 0), stop=(k == 4))
        ysb = ypool.tile([128, 512], F32, tag="ysb")
        if t % 2 == 0:
            nc.vector.tensor_copy(out=ysb, in_=ypsum)
        else:
            nc.scalar.activation(out=ysb, in_=ypsum, func=AF.Copy,
                                 bias=0.0, scale=1.0)
        nc.sync.dma_start(out=y_out[128 * t:128 * (t + 1), :], in_=ysb)
    ctx.close()


def make_core_inputs(core, x, ln_gamma, ln_beta, w_in, w_out):
    """Host-side slicing for one core. Returns the in_map dict."""
    b, r = core // 4, core % 4
    heads = (r, r + 4)

    g = np.asarray(ln_gamma, np.float32)
    be = np.asarray(ln_beta, np.float32)
    wi = np.asarray(w_in, np.float32).copy()
    # fold spm post-scale: spm_true = 0.2*ln(cumsum) + 3  (shift 15 folded)
    wi_spm = wi[SPM_LO:SPM_HI, :].copy()
    bias_full = 3.0 * wi_spm.sum(0)          # [3584]
    # fold gamma/beta of the non-spm channels into w_in / bias (the kernel's
    # xt holds raw normalized activations; spm applies gamma via g5b5)
    nonspm = np.ones(D, bool); nonspm[SPM_LO:SPM_HI] = False
    bias_full += (be[nonspm, None] * wi[nonspm, :]).sum(0)
    # shift channels contribute no beta at padded positions (corrected below)
    e2 = (be[448:512, None] * wi[448:512, :]).sum(0)
    e1 = (be[384:448, None] * wi[384:448, :]).sum(0) + e2
    wi[nonspm, :] *= g[nonspm, None]
    wi[SPM_LO:SPM_HI, :] = 0.2 * wi_spm

    cols = []
    for part in range(3):                     # q, k, v
        for h in heads:
            c0 = part * D + h * DH
            cols.extend(range(c0, c0 + DH))
    c0 = 3 * D + r * PSLICE
    cols.extend(range(c0, c0 + PSLICE))
    cols = np.array(cols)
    w_in_s = wi[:, cols].astype(nbf)          # [512, 896]
    bias_s = bias_full[cols].astype(np.float32)
    biasin = bias_s.reshape(7, 128).T.copy()  # [128, 7]
    edgec = np.zeros((128, 14), np.float32)
    e1s, e2s = e1[cols].reshape(7, 128), e2[cols].reshape(7, 128)
    for o in range(7):
        edgec[:, 2 * o] = e1s[o]
        edgec[:, 2 * o + 1] = e2s[o]

    rows = []
    for h in heads:
        rows.extend(range(h * DH, (h + 1) * DH))
    rows.extend(range(D + r * PSLICE, D + (r + 1) * PSLICE))
    w_out_s = np.asarray(w_out, np.float32)[np.array(rows), :].astype(nbf)

    qaug = np.stack([-8.0 * SLOPES[heads[0]] * np.arange(L, dtype=np.float32),
                     -8.0 * SLOPES[heads[1]] * np.arange(L, dtype=np.float32),
                     np.ones(L, np.float32)]).astype(nbf)
    expb = np.zeros((128, 32), np.float32)
    jj = np.arange(128, dtype=np.float32)
    for s, h in enumerate(heads):
        for jb in range(16):
            expb[:, 16 * s + jb] = SLOPES[h] * (128 * jb + jj)

    g5b5 = np.stack([5.0 * g[SPM_LO:SPM_HI],
                     5.0 * be[SPM_LO:SPM_HI] - 15.0], 1)

    tri = (np.arange(128)[:, None] <= np.arange(128)[None, :])
    return {
        "x": np.ascontiguousarray(np.asarray(x, np.float32)[b]),
        "w_in": w_in_s,
        "w_out": w_out_s,
        "qaug": qaug,
        "expb": expb,
        "biasin": biasin,
        "edgec": edgec,
        "g5b5": g5b5.astype(np.float32),
        "tri01": tri.astype(nbf),
    }


_CACHED = {}


def kernel(x, ln_gamma, ln_beta, w_in, w_out, b_out):
    if "nc" not in _CACHED:
        _CACHED["nc"] = build_program()
    nc = _CACHED["nc"]
    in_maps = [make_core_inputs(c, x, ln_gamma, ln_beta, w_in, w_out)
               for c in range(8)]
    res = run_bass_kernel_spmd(nc, in_maps, core_ids=list(range(8)))
    parts = [res.results[c]["y"] for c in range(8)]
    bo = np.asarray(b_out, np.float32)
    out = np.stack([
        parts[0] + parts[1] + parts[2] + parts[3] + bo,
        parts[4] + parts[5] + parts[6] + parts[7] + bo,
    ]).astype(np.float32)
    return out
